# revision 7
# baseline (speedup 1.0000x reference)
"""AdaptiveRankingLoss distributed Bass kernel for 8 TRN2 NeuronCores (v2).

Math
----
loss = sum_{i<j, t_i != t_j} w_ij * relu(m_ij - sign(t_i-t_j)*(p_i-p_j)) / count
  m = 0.1*clip(|t_i-t_j|, 0.1, 1.0),  w = 1/(1+u_i+u_j).

Host sorts by t; with a = t_j - t_i (sorted ascending) the full-matrix
summand [a>0] * w * relu(clip(0.1a, .01, .1) - (p_j - p_i)) is split into
t-distance BANDS with cheap per-band formulas:

  far  (a >= 1):      relu(0.1  - (p_j - p_i)) = relu(pn_j + (p_i + 0.1))
  mid  (0.1<=a<=1):   relu(0.1a - (p_j - p_i)) = relu(q_j  + (-q_i))
  near (0<a<0.1):     relu(0.01 - (p_j - p_i)) = relu(pn_j + (p_i + 0.01))
  mixed/indicator:    full custom 8-stage DVE op
with pn = -p, q = 0.1t - p (both fp16 broadcast columns). Pure-band runs
use plain TENSOR_SCALAR (add, max0) with per-partition fp32 scalars at
~3.7x the custom op's throughput; a few large far runs go to the (idle)
scalar/ACT engine as relu(1*pn + bias).

Band boundaries are band-uniform (computed from each 1024-row band's
t-range) so the single SPMD program is valid for every core: core c owns
rows {1024k + 8p + c} (strided within band), and each band's zone edges
cover all cores' rows.

Weights via the degree-6 bilinear split  w ~ sum_n Phi_n(x_i) Psi_n(x_j)
(x = u - 0.5): v-tiles are contracted over the 128 row-partitions by the
TensorEngine into X[n, j] (PSUM, 16 x [7,512] chunks over 8 banks, two
column phases), then sum_j X[n,j] Psi_n(j) via DVE tensor-tensor-reduce,
mostly on a DMA-partition-reshaped [112, .] layout. Host sums the 8
cores' accumulators and divides by the exact (tie-aware) pair count.
"""

import numpy as np

import concourse.bass as bass
import concourse.bacc as bacc
import concourse.mybir as mybir
import concourse.tile as tile
from concourse.bass_utils import run_bass_kernel_spmd
from concourse import dve_ops
from concourse.dve_spec import (
    Spec,
    Src0,
    Src1,
    C0,
    C1,
    C2,
    Zero,
    relu,
    maxx,
    minn,
    lower,
    _has_src1,
)
from concourse.dve_uop import DveOpSpec

F32 = mybir.dt.float32
F16 = mybir.dt.float16
BF16 = mybir.dt.bfloat16
AL = mybir.AluOpType

N = 8192
NCORES = 8
P = 128
NSLOT = 8          # 1024-row bands
K = 7              # weight polynomial terms
CH = 1024          # psum chunk width (2 banks)
NCHUNK = N // CH   # 8
PHASE_SPLIT = 4096
MIN_RUN = 384      # pure runs narrower than this merge into custom


# --------------------------------------------------------------------------
# custom DVE op with NEGATED p column:
#   a = Src0 - C0;  v = [a>0] * relu(clip(a, C2^2, C2) + Src1 + C1)
#   Src0 = 0.1*t_col (f16), C0 = 0.1*t_row, Src1 = -p_col (f16), C1 = p_row.
# --------------------------------------------------------------------------
_ARL_NAME = "ARL_PN_V2"


def _arl_reference(in0, in1, s0, s1, imm2):
    a = in0 - s0
    m = np.clip(a, np.float32(imm2) * np.float32(imm2), imm2)
    return (a > 0).astype(np.float32) * np.maximum(m + in1 + s1, 0.0)


def _register_arl_op():
    for op in dve_ops.OPS:
        if op.name == _ARL_NAME:
            return op
    a = Src0 - C0
    m = minn(maxx(a, C2 * C2), C2)
    h = relu((m + Src1) + C1)
    spec = Spec(body=(a > Zero) * h, reference=_arl_reference)
    row = dve_ops._CUSTOM_DVE_ROW_BASE + len(dve_ops.OPS)
    assert row < 0x20, "custom-DVE row overflow"
    dve_ops._SUB_OPCODE_FOR_NAME[_ARL_NAME] = row
    shas = {}
    for ver in ("v3", "v4"):
        try:
            uops = lower(spec, ver=ver)
            shas[ver] = DveOpSpec(
                name=_ARL_NAME, opcode=row, uops=uops, rd1_en=_has_src1(spec)
            ).sha(ver)
        except Exception:
            pass
    op = dve_ops.DveOp(_ARL_NAME, spec, subdim=False, uops_sha=shas)
    dve_ops.OPS.append(op)
    dve_ops.CUSTOM_DVE_SPECS[_ARL_NAME] = spec
    return op


ARL_PN = _register_arl_op()


# --------------------------------------------------------------------------
# degree-6 bilinear split of w = 1/(1+u_i+u_j) = 1/(2 + x_i + x_j), x = u-.5
# --------------------------------------------------------------------------
def _acoef_matrix() -> np.ndarray:
    from numpy.polynomial import chebyshev as _C
    from math import comb

    deg = K - 1
    nodes = np.cos((2 * np.arange(deg + 1) + 1) / (2 * (deg + 1)) * np.pi)
    ch = _C.chebfit(nodes, 1.0 / (2.0 + nodes), deg)
    c = _C.cheb2poly(ch)
    A = np.zeros((K, K), np.float64)
    for mm in range(K):
        for nn in range(K):
            if mm + nn <= deg:
                A[mm, nn] = c[mm + nn] * comb(mm + nn, mm)
    return A  # float64


_ACOEF = _acoef_matrix()


# --------------------------------------------------------------------------
# plan: band zones + run lists from the sorted targets (host side)
# --------------------------------------------------------------------------
def _make_plan(ts32: np.ndarray):
    ts = ts32.astype(np.float64)
    slot_runs = []  # per slot: list of (kind, a, b) over full column range
    for k in range(NSLOT):
        t_lo = ts[1024 * k]
        t_hi = ts[1024 * k + 1023]
        jA = 1024 * k  # extend down to band start (extra cols give exact 0)
        B1 = int(np.searchsorted(ts, t_hi + 0.1, "left"))
        B2 = int(np.searchsorted(ts, t_lo + 1.0, "right"))
        B3 = int(np.searchsorted(ts, t_hi + 1.0, "left"))
        B1 = max(B1, jA)
        B2 = max(B2, B1)
        B3 = max(B3, B2)
        runs = []
        if B1 > jA:
            runs.append(["cust", jA, B1])
        if B2 > B1:
            runs.append(["mid", B1, B2])
        if B3 > B2:
            runs.append(["cust", B2, B3])
        if N > B3:
            runs.append(["far", B3, N])
        # merge tiny pure runs into custom neighbours
        changed = True
        while changed:
            changed = False
            for i, r in enumerate(runs):
                if r[0] != "cust" and r[2] - r[1] < MIN_RUN:
                    r[0] = "cust"
                    changed = True
            i = 0
            while i + 1 < len(runs):
                if runs[i][0] == "cust" and runs[i + 1][0] == "cust":
                    runs[i][2] = runs[i + 1][2]
                    del runs[i + 1]
                    changed = True
                else:
                    i += 1
        slot_runs.append([(r[0], r[1], r[2]) for r in runs])

    # phase-clipped emissions
    def clip_runs(runs, lo, hi):
        out = []
        for kind, a, b in runs:
            a2, b2 = max(a, lo), min(b, hi)
            if b2 > a2:
                out.append((kind, a2, b2))
        return out

    emitA = [clip_runs(slot_runs[k], PHASE_SPLIT, N) for k in range(NSLOT)]
    emitB = [clip_runs(slot_runs[k], 0, PHASE_SPLIT) for k in range(NSLOT)]

    # split slot0's phase-A far run for a faster PE start
    if emitA[0] and emitA[0][0][0] == "far" and emitA[0][0][2] - emitA[0][0][1] > 2048:
        kind, a, b = emitA[0][0]
        cuts = [a, a + 512, a + 1024, a + 2048, b]
        emitA[0] = [(kind, x, y) for x, y in zip(cuts, cuts[1:])] + emitA[0][1:]

    # chunk first/last slot tables per phase
    def chunk_tables(emits, lo, hi):
        first = {}
        last = {}
        for k in range(NSLOT):
            for kind, a, b in emits[k]:
                c0, c1 = a // CH, (b - 1) // CH
                for c in range(c0, c1 + 1):
                    if c not in first:
                        first[c] = k
                    last[c] = k
        return first, last

    firstA, lastA = chunk_tables(emitA, PHASE_SPLIT, N)
    firstB, lastB = chunk_tables(emitB, 0, PHASE_SPLIT)

    # column ranges needed per tensor (rounded out to 256)
    def ranges_for(kinds):
        need = np.zeros(N, bool)
        for k in range(NSLOT):
            for kind, a, b in slot_runs[k]:
                if kind in kinds:
                    need[a:b] = True
        out = []
        j = 0
        while j < N:
            if need[j]:
                e = j
                while e < N and need[e]:
                    e += 1
                a = (j // 256) * 256
                b = min(N, ((e + 255) // 256) * 256)
                if out and a <= out[-1][1]:
                    out[-1] = (out[-1][0], b)
                else:
                    out.append((a, b))
                j = e
            else:
                j += 1
        return out

    t01_ranges = ranges_for(("cust",))
    q_ranges = ranges_for(("mid",))
    return {
        "emitA": emitA,
        "emitB": emitB,
        "firstA": firstA,
        "lastA": lastA,
        "firstB": firstB,
        "lastB": lastB,
        "t01_ranges": t01_ranges,
        "q_ranges": q_ranges,
    }


# --------------------------------------------------------------------------
# device graph
# --------------------------------------------------------------------------
# scalar slots in rows5: 0: 0.1*t  1: p  2: -q  3: p+0.01  4: p+0.1
_KIND_SCAL = {"mid": 2, "near": 3, "far": 4}
NACC = 8  # accumulator columns


def _build_nc(plan):
    from contextlib import ExitStack

    nc = bacc.Bacc(None, target_bir_lowering=False, debug=False)

    t01_ext = nc.declare_dram_parameter("t01col", [N], F16, isOutput=False)
    pn_ext = nc.declare_dram_parameter("pncol", [N], F16, isOutput=False)
    q_ext = nc.declare_dram_parameter("qcol", [N], F16, isOutput=False)
    rows_ext = nc.declare_dram_parameter("rows5", [P, 5 * NSLOT], F32, isOutput=False)
    phib_ext = nc.declare_dram_parameter("phib", [P, NSLOT * K], BF16, isOutput=False)
    psi_ext = nc.declare_dram_parameter("psi", [K, N], F32, isOutput=False)
    psiRA_ext = nc.declare_dram_parameter("psiRA", [K * 16, 256], F32, isOutput=False)
    psiRB_ext = nc.declare_dram_parameter("psiRB", [K * 16, 128], F32, isOutput=False)
    out_ext = nc.declare_dram_parameter("out", [K * 16 * NACC], F32, isOutput=True)

    emitA, emitB = plan["emitA"], plan["emitB"]
    firstA, lastA = plan["firstA"], plan["lastA"]
    firstB, lastB = plan["firstB"], plan["lastB"]

    with tile.TileContext(nc) as tc, ExitStack() as ctx:
        constp = ctx.enter_context(tc.tile_pool(name="const", bufs=1))
        colp = ctx.enter_context(tc.tile_pool(name="cols", bufs=1))
        vp = ctx.enter_context(tc.tile_pool(name="v", bufs=4))
        pp = ctx.enter_context(tc.tile_pool(name="psum", bufs=4, space="PSUM"))
        sp = ctx.enter_context(tc.tile_pool(name="small", bufs=1))

        t01_sb = colp.tile([P, N], F16)
        pn_sb = colp.tile([P, N], F16)
        q_sb = colp.tile([P, N], F16)

        def bcast(dst, ext, lo, hi, piece=4096):
            while lo < hi:
                m = min(lo + piece, hi)
                nc.sync.dma_start(
                    dst[:, lo:m],
                    bass.AP(tensor=ext, offset=lo, ap=[[0, P], [1, m - lo]]),
                )
                lo = m

        # small per-partition tensors first (they gate the first runs and
        # their many tiny DMA packets are cheap to front-run), then column
        # broadcasts in consumption order. Broadcast pieces are kept large
        # (>=2048 cols = 4KB packets) for DMA-queue throughput, with two
        # small lead pieces so the first runs can start early.
        rows_sb = constp.tile([P, NSLOT, 5], F32)
        nc.sync.dma_start(
            rows_sb[:], rows_ext[:, :].rearrange("p (r s) -> p r s", s=5)
        )
        phib = constp.tile([P, NSLOT, K], BF16)
        nc.sync.dma_start(
            phib[:], phib_ext[:, :].rearrange("p (r k) -> p r k", k=K)
        )
        bcast(pn_sb, pn_ext, 4096, 4608, piece=512)
        bcast(pn_sb, pn_ext, 4608, 5632, piece=1024)
        bcast(pn_sb, pn_ext, 5632, 8192, piece=2560)
        for lo, hi in plan["t01_ranges"]:
            if hi > 4096:
                bcast(t01_sb, t01_ext, max(lo, 4096), hi, piece=4096)
        for lo, hi in plan["q_ranges"]:
            if hi > 4096:
                bcast(q_sb, q_ext, max(lo, 4096), hi, piece=4096)
        bcast(pn_sb, pn_ext, 0, 4096, piece=2048)
        for lo, hi in plan["t01_ranges"]:
            if lo < 4096:
                bcast(t01_sb, t01_ext, lo, min(hi, 4096), piece=4096)
        for lo, hi in plan["q_ranges"]:
            if lo < 4096:
                bcast(q_sb, q_ext, lo, min(hi, 4096), piece=4096)
        psi = constp.tile([K, N], F32)
        nc.sync.dma_start(psi[:], psi_ext[:, :])
        psiRA = constp.tile([K * 16, 256], F32)
        nc.sync.dma_start(psiRA[:], psiRA_ext[:, :])
        psiRB = constp.tile([K * 16, 128], F32)
        nc.sync.dma_start(psiRB[:], psiRB_ext[:, :])

        acc = sp.tile([K * 16, NACC], F32)
        nc.vector.memset(acc[:], 0.0)
        XsA = sp.tile([K, 4096], F32)
        XsB = sp.tile([K, 2048], F32)
        ttr_scr = sp.tile([K * 16, 256], F32)
        ttr_scr2 = sp.tile([K, CH], F32)

        chunk_tiles = {}

        def get_chunk(c):
            if c not in chunk_tiles:
                chunk_tiles[c] = pp.tile([K, CH], F32, tag="X", name=f"X{c}")
            return chunk_tiles[c]

        # ACT offload set: phase-A far runs of slots 1..3
        act_far = set()
        for k in (1, 2, 3):
            for kind, a, b in emitA[k]:
                if kind == "far":
                    act_far.add((k, a, b))

        def emit_run(k, kind, a, b, first_t, last_t):
            while b - a > 4096:
                emit_run(k, kind, a, a + 4096, first_t, last_t)
                a += 4096
            w = b - a
            v = vp.tile([P, 4096], BF16, tag="v")
            if kind == "cust":
                nc.vector._custom_dve(
                    ARL_PN,
                    out=v[:, :w],
                    in0=t01_sb[:, a:b],
                    in1=pn_sb[:, a:b],
                    s0=rows_sb[:, k, 0:1],
                    s1=rows_sb[:, k, 1:2],
                    imm2=0.1,
                )
            elif (k, a, b) in act_far:
                nc.scalar.activation(
                    v[:, :w],
                    pn_sb[:, a:b],
                    mybir.ActivationFunctionType.Relu,
                    bias=rows_sb[:, k, 4:5],
                    scale=1.0,
                )
            else:
                src = q_sb if kind == "mid" else pn_sb
                nc.vector.tensor_scalar(
                    v[:, :w],
                    src[:, a:b],
                    rows_sb[:, k, _KIND_SCAL[kind] : _KIND_SCAL[kind] + 1],
                    0.0,
                    AL.add,
                    AL.max,
                )
            c0, c1 = a // CH, (b - 1) // CH
            for c in range(c0, c1 + 1):
                lo, hi = max(a, c * CH), min(b, (c + 1) * CH)
                for m0 in range(lo, hi, 512):
                    m1 = min(m0 + 512, hi)
                    nc.tensor.matmul(
                        get_chunk(c)[:, m0 - c * CH : m1 - c * CH],
                        phib[:, k, :],
                        v[:, m0 - a : m1 - a],
                        start=(k == first_t[c]),
                        stop=(k == last_t[c]),
                    )

        def drain_copy(c, Xs, base):
            nc.scalar.copy(
                Xs[:, c * CH - base : (c + 1) * CH - base], get_chunk(c)[:]
            )

        def drain_direct(c, slot_idx):
            nc.vector._custom_dve(
                dve_ops.TENSOR_TENSOR_REDUCE,
                out=ttr_scr2[:],
                in0=get_chunk(c)[:],
                in1=psi[:, c * CH : (c + 1) * CH],
                s0=0.0,
                s1=1.0,
                accum_out=acc[0:K, slot_idx : slot_idx + 1],
            )

        # ---- phase A: columns [4096, 8192) ----
        drainedA = []
        for k in range(NSLOT):
            for kind, a, b in emitA[k]:
                emit_run(k, kind, a, b, firstA, lastA)
            for c in sorted(lastA):
                if lastA[c] == k:
                    drain_copy(c, XsA, PHASE_SPLIT)
                    drainedA.append(c)
        # reshape [7,4096] -> [112,256] (per-n DMAs); TTR emitted later so
        # the in-order DVE queue does not stall on the drain dependencies.
        xrA = sp.tile([K * 16, 256], F32)
        for n in range(K):
            nc.sync.dma_start(
                xrA[n * 16 : (n + 1) * 16, :],
                XsA[n : n + 1, :].rearrange("o (k f) -> o k f", k=16),
            )

        # ---- phase B: columns [0, 4096) ----
        xrB = sp.tile([K * 16, 128], F32)
        ttr_scrB = sp.tile([K * 16, 128], F32)
        for k in range(NSLOT):
            for kind, a, b in emitB[k]:
                emit_run(k, kind, a, b, firstB, lastB)
            for c in sorted(lastB):
                if lastB[c] == k:
                    if c <= 1:
                        drain_copy(c, XsB, 0)
                    else:
                        drain_direct(c, c)
            if k == 1:
                # xrB reshape can dispatch once chunks 0-1 are copied
                for n in range(K):
                    nc.sync.dma_start(
                        xrB[n * 16 : (n + 1) * 16, :],
                        XsB[n : n + 1, :].rearrange("o (k f) -> o k f", k=16),
                    )
            if k == 2:
                nc.vector._custom_dve(
                    dve_ops.TENSOR_TENSOR_REDUCE,
                    out=ttr_scr[:],
                    in0=xrA[:],
                    in1=psiRA[:],
                    s0=0.0,
                    s1=1.0,
                    accum_out=acc[:, 0:1],
                )
        nc.vector._custom_dve(
            dve_ops.TENSOR_TENSOR_REDUCE,
            out=ttr_scrB[:],
            in0=xrB[:],
            in1=psiRB[:],
            s0=0.0,
            s1=1.0,
            accum_out=acc[:, 1:2],
        )

        nc.sync.dma_start(
            out_ext[:].rearrange("(p c) -> p c", c=NACC), acc[:]
        )

    nc.compile()
    return nc


_NC_CACHE = {}


def _get_nc(plan, key):
    if key not in _NC_CACHE:
        _NC_CACHE[key] = _build_nc(plan)
    return _NC_CACHE[key]


def _exact_count(t: np.ndarray) -> int:
    n = t.shape[0]
    _, cnts = np.unique(t, return_counts=True)
    dup = int(sum(int(c) * (int(c) - 1) // 2 for c in cnts[cnts > 1]))
    return n * (n - 1) // 2 - dup


def _make_in_maps(predictions, targets, uncertainties):
    import ml_dtypes

    t = np.ascontiguousarray(np.asarray(targets, np.float32))
    p = np.ascontiguousarray(np.asarray(predictions, np.float32))
    u = np.ascontiguousarray(np.asarray(uncertainties, np.float32))
    order = np.argsort(t, kind="stable")
    ts, ps, us = t[order], p[order], u[order]
    ts64, ps64 = ts.astype(np.float64), ps.astype(np.float64)

    t01_h = (0.1 * ts64).astype(np.float16)
    pn_h = (-ps64).astype(np.float16)
    q_h = (0.1 * ts64 - ps64).astype(np.float16)

    # Psi[n, j] = x_j^n, x = u - 0.5
    x = us.astype(np.float64) - 0.5
    psi = np.stack([x**n for n in range(K)]).astype(np.float32)  # [K, N]
    psiRA = np.ascontiguousarray(
        psi[:, PHASE_SPLIT:].reshape(K, 16, 256).reshape(K * 16, 256)
    )
    psiRB = np.ascontiguousarray(
        psi[:, 0:2048].reshape(K, 16, 128).reshape(K * 16, 128)
    )

    # Phi[i, n] = sum_m A[m, n] x_i^m  (float64 -> bf16)
    xp = np.stack([x**m for m in range(K)])  # [K, N]
    phi = np.einsum("mn,mj->jn", _ACOEF, xp)  # [N, K]

    in_maps = []
    for c in range(NCORES):
        pos = (np.arange(NSLOT)[:, None] * 1024 + 8 * np.arange(P)[None, :] + c)
        # rows5[p, slot, s]
        rows5 = np.zeros((P, NSLOT, 5), np.float64)
        tp = ts64[pos]  # [slot, p]
        pp_ = ps64[pos]
        rows5[:, :, 0] = (0.1 * tp).T
        rows5[:, :, 1] = pp_.T
        rows5[:, :, 2] = (-(0.1 * tp - pp_)).T
        rows5[:, :, 3] = (pp_ + 0.01).T
        rows5[:, :, 4] = (pp_ + 0.1).T
        phib = phi[pos, :]  # [slot, p, K]
        phib = np.ascontiguousarray(np.transpose(phib, (1, 0, 2)))  # [p, slot, K]
        in_maps.append(
            {
                "t01col": t01_h,
                "pncol": pn_h,
                "qcol": q_h,
                "rows5": np.ascontiguousarray(
                    rows5.reshape(P, 5 * NSLOT)
                ).astype(np.float32),
                "phib": phib.reshape(P, NSLOT * K).astype(ml_dtypes.bfloat16),
                "psi": psi,
                "psiRA": psiRA,
                "psiRB": psiRB,
            }
        )
    plan = _make_plan(ts)
    return in_maps, t, plan


def _run_device(in_maps, plan, plan_key, trace=False, **kw):
    nc = _get_nc(plan, plan_key)
    return run_bass_kernel_spmd(
        nc, in_maps, core_ids=list(range(NCORES)), trace=trace, **kw
    )


def _plan_key(plan):
    return (
        tuple(tuple(r) for k in range(NSLOT) for r in plan["emitA"][k]),
        tuple(tuple(r) for k in range(NSLOT) for r in plan["emitB"][k]),
    )


def kernel(predictions, targets, uncertainties):
    in_maps, t, plan = _make_in_maps(predictions, targets, uncertainties)
    res = _run_device(in_maps, plan, _plan_key(plan))
    total = np.float64(0.0)
    for r in res.results:
        total += np.asarray(r["out"], np.float64).sum()
    count = _exact_count(t)
    return np.asarray(total / max(count, 1), dtype=np.float32)


# revision 8
# speedup vs baseline: 1.0437x; 1.0437x over previous
"""AdaptiveRankingLoss distributed Bass kernel for 8 TRN2 NeuronCores (v2).

Math
----
loss = sum_{i<j, t_i != t_j} w_ij * relu(m_ij - sign(t_i-t_j)*(p_i-p_j)) / count
  m = 0.1*clip(|t_i-t_j|, 0.1, 1.0),  w = 1/(1+u_i+u_j).

Host sorts by t; with a = t_j - t_i (sorted ascending) the full-matrix
summand [a>0] * w * relu(clip(0.1a, .01, .1) - (p_j - p_i)) is split into
t-distance BANDS with cheap per-band formulas:

  far  (a >= 1):      relu(0.1  - (p_j - p_i)) = relu(pn_j + (p_i + 0.1))
  mid  (0.1<=a<=1):   relu(0.1a - (p_j - p_i)) = relu(q_j  + (-q_i))
  near (0<a<0.1):     relu(0.01 - (p_j - p_i)) = relu(pn_j + (p_i + 0.01))
  mixed/indicator:    full custom 8-stage DVE op
with pn = -p, q = 0.1t - p (both fp16 broadcast columns). Pure-band runs
use plain TENSOR_SCALAR (add, max0) with per-partition fp32 scalars at
~3.7x the custom op's throughput; a few large far runs go to the (idle)
scalar/ACT engine as relu(1*pn + bias).

Band boundaries are band-uniform (computed from each 1024-row band's
t-range) so the single SPMD program is valid for every core: core c owns
rows {1024k + 8p + c} (strided within band), and each band's zone edges
cover all cores' rows.

Weights via the degree-6 bilinear split  w ~ sum_n Phi_n(x_i) Psi_n(x_j)
(x = u - 0.5): v-tiles are contracted over the 128 row-partitions by the
TensorEngine into X[n, j] (PSUM, 16 x [7,512] chunks over 8 banks, two
column phases), then sum_j X[n,j] Psi_n(j) via DVE tensor-tensor-reduce,
mostly on a DMA-partition-reshaped [112, .] layout. Host sums the 8
cores' accumulators and divides by the exact (tie-aware) pair count.
"""

import numpy as np

import concourse.bass as bass
import concourse.bacc as bacc
import concourse.mybir as mybir
import concourse.tile as tile
from concourse.bass_utils import run_bass_kernel_spmd
from concourse import dve_ops
from concourse.dve_spec import (
    Spec,
    Src0,
    Src1,
    C0,
    C1,
    C2,
    Zero,
    relu,
    maxx,
    minn,
    lower,
    _has_src1,
)
from concourse.dve_uop import DveOpSpec

F32 = mybir.dt.float32
F16 = mybir.dt.float16
BF16 = mybir.dt.bfloat16
AL = mybir.AluOpType

N = 8192
NCORES = 8
P = 128
NSLOT = 8          # 1024-row bands
K = 7              # weight polynomial terms
CH = 1024          # psum chunk width (2 banks)
NCHUNK = N // CH   # 8
PHASE_SPLIT = 4096
MIN_RUN = 384      # pure runs narrower than this merge into custom


# --------------------------------------------------------------------------
# custom DVE op with NEGATED p column:
#   a = Src0 - C0;  v = [a>0] * relu(clip(a, C2^2, C2) + Src1 + C1)
#   Src0 = 0.1*t_col (f16), C0 = 0.1*t_row, Src1 = -p_col (f16), C1 = p_row.
# --------------------------------------------------------------------------
_ARL_NAME = "ARL_PN_V2"


def _arl_reference(in0, in1, s0, s1, imm2):
    a = in0 - s0
    m = np.clip(a, np.float32(imm2) * np.float32(imm2), imm2)
    return (a > 0).astype(np.float32) * np.maximum(m + in1 + s1, 0.0)


def _register_arl_op():
    for op in dve_ops.OPS:
        if op.name == _ARL_NAME:
            return op
    a = Src0 - C0
    m = minn(maxx(a, C2 * C2), C2)
    h = relu((m + Src1) + C1)
    spec = Spec(body=(a > Zero) * h, reference=_arl_reference)
    row = dve_ops._CUSTOM_DVE_ROW_BASE + len(dve_ops.OPS)
    assert row < 0x20, "custom-DVE row overflow"
    dve_ops._SUB_OPCODE_FOR_NAME[_ARL_NAME] = row
    shas = {}
    for ver in ("v3", "v4"):
        try:
            uops = lower(spec, ver=ver)
            shas[ver] = DveOpSpec(
                name=_ARL_NAME, opcode=row, uops=uops, rd1_en=_has_src1(spec)
            ).sha(ver)
        except Exception:
            pass
    op = dve_ops.DveOp(_ARL_NAME, spec, subdim=False, uops_sha=shas)
    dve_ops.OPS.append(op)
    dve_ops.CUSTOM_DVE_SPECS[_ARL_NAME] = spec
    return op


ARL_PN = _register_arl_op()


# --------------------------------------------------------------------------
# degree-6 bilinear split of w = 1/(1+u_i+u_j) = 1/(2 + x_i + x_j), x = u-.5
# --------------------------------------------------------------------------
def _acoef_matrix() -> np.ndarray:
    from numpy.polynomial import chebyshev as _C
    from math import comb

    deg = K - 1
    nodes = np.cos((2 * np.arange(deg + 1) + 1) / (2 * (deg + 1)) * np.pi)
    ch = _C.chebfit(nodes, 1.0 / (2.0 + nodes), deg)
    c = _C.cheb2poly(ch)
    A = np.zeros((K, K), np.float64)
    for mm in range(K):
        for nn in range(K):
            if mm + nn <= deg:
                A[mm, nn] = c[mm + nn] * comb(mm + nn, mm)
    return A  # float64


_ACOEF = _acoef_matrix()


# --------------------------------------------------------------------------
# plan: band zones + run lists from the sorted targets (host side)
# --------------------------------------------------------------------------
def _make_plan(ts32: np.ndarray):
    ts = ts32.astype(np.float64)
    slot_runs = []  # per slot: list of (kind, a, b) over full column range
    for k in range(NSLOT):
        t_lo = ts[1024 * k]
        t_hi = ts[1024 * k + 1023]
        jA = 1024 * k  # extend down to band start (extra cols give exact 0)
        B1 = int(np.searchsorted(ts, t_hi + 0.1, "left"))
        B2 = int(np.searchsorted(ts, t_lo + 1.0, "right"))
        B3 = int(np.searchsorted(ts, t_hi + 1.0, "left"))
        B1 = max(B1, jA)
        B2 = max(B2, B1)
        B3 = max(B3, B2)
        runs = []
        if B1 > jA:
            runs.append(["cust", jA, B1])
        if B2 > B1:
            runs.append(["mid", B1, B2])
        if B3 > B2:
            runs.append(["cust", B2, B3])
        if N > B3:
            runs.append(["far", B3, N])
        # merge tiny pure runs into custom neighbours
        changed = True
        while changed:
            changed = False
            for i, r in enumerate(runs):
                if r[0] != "cust" and r[2] - r[1] < MIN_RUN:
                    r[0] = "cust"
                    changed = True
            i = 0
            while i + 1 < len(runs):
                if runs[i][0] == "cust" and runs[i + 1][0] == "cust":
                    runs[i][2] = runs[i + 1][2]
                    del runs[i + 1]
                    changed = True
                else:
                    i += 1
        slot_runs.append([(r[0], r[1], r[2]) for r in runs])

    # phase-clipped emissions
    def clip_runs(runs, lo, hi):
        out = []
        for kind, a, b in runs:
            a2, b2 = max(a, lo), min(b, hi)
            if b2 > a2:
                out.append((kind, a2, b2))
        return out

    emitA = [clip_runs(slot_runs[k], PHASE_SPLIT, N) for k in range(NSLOT)]
    emitB = [clip_runs(slot_runs[k], 0, PHASE_SPLIT) for k in range(NSLOT)]

    # split slot0's phase-A far run for a faster PE start
    if emitA[0] and emitA[0][0][0] == "far" and emitA[0][0][2] - emitA[0][0][1] > 2048:
        kind, a, b = emitA[0][0]
        cuts = [a, a + 512, a + 1024, a + 2048, b]
        emitA[0] = [(kind, x, y) for x, y in zip(cuts, cuts[1:])] + emitA[0][1:]

    # chunk first/last slot tables per phase
    def chunk_tables(emits, lo, hi):
        first = {}
        last = {}
        for k in range(NSLOT):
            for kind, a, b in emits[k]:
                c0, c1 = a // CH, (b - 1) // CH
                for c in range(c0, c1 + 1):
                    if c not in first:
                        first[c] = k
                    last[c] = k
        return first, last

    firstA, lastA = chunk_tables(emitA, PHASE_SPLIT, N)
    firstB, lastB = chunk_tables(emitB, 0, PHASE_SPLIT)

    # column ranges needed per tensor (rounded out to 256)
    def ranges_for(kinds):
        need = np.zeros(N, bool)
        for k in range(NSLOT):
            for kind, a, b in slot_runs[k]:
                if kind in kinds:
                    need[a:b] = True
        out = []
        j = 0
        while j < N:
            if need[j]:
                e = j
                while e < N and need[e]:
                    e += 1
                a = (j // 256) * 256
                b = min(N, ((e + 255) // 256) * 256)
                if out and a <= out[-1][1]:
                    out[-1] = (out[-1][0], b)
                else:
                    out.append((a, b))
                j = e
            else:
                j += 1
        return out

    t01_ranges = ranges_for(("cust",))
    q_ranges = ranges_for(("mid",))
    return {
        "emitA": emitA,
        "emitB": emitB,
        "firstA": firstA,
        "lastA": lastA,
        "firstB": firstB,
        "lastB": lastB,
        "t01_ranges": t01_ranges,
        "q_ranges": q_ranges,
    }


# --------------------------------------------------------------------------
# device graph
# --------------------------------------------------------------------------
# scalar slots in rows5: 0: 0.1*t  1: p  2: -q  3: p+0.01  4: p+0.1
_KIND_SCAL = {"mid": 2, "near": 3, "far": 4}
NACC = 8  # accumulator columns


def _build_nc(plan):
    from contextlib import ExitStack

    nc = bacc.Bacc(None, target_bir_lowering=False, debug=False)

    t01_ext = nc.declare_dram_parameter("t01col", [N], F16, isOutput=False)
    pn_ext = nc.declare_dram_parameter("pncol", [N], F16, isOutput=False)
    q_ext = nc.declare_dram_parameter("qcol", [N], F16, isOutput=False)
    rows_ext = nc.declare_dram_parameter("rows5", [P, 5 * NSLOT], F32, isOutput=False)
    phib_ext = nc.declare_dram_parameter("phib", [P, NSLOT * K], BF16, isOutput=False)
    psi_ext = nc.declare_dram_parameter("psi", [K, N], F32, isOutput=False)
    psiRA_ext = nc.declare_dram_parameter("psiRA", [K * 16, 256], F32, isOutput=False)
    psiRB_ext = nc.declare_dram_parameter("psiRB", [K * 16, 128], F32, isOutput=False)
    out_ext = nc.declare_dram_parameter("out", [K * 16 * NACC], F32, isOutput=True)

    emitA, emitB = plan["emitA"], plan["emitB"]
    firstA, lastA = plan["firstA"], plan["lastA"]
    firstB, lastB = plan["firstB"], plan["lastB"]

    with tile.TileContext(nc) as tc, ExitStack() as ctx:
        constp = ctx.enter_context(tc.tile_pool(name="const", bufs=1))
        colp = ctx.enter_context(tc.tile_pool(name="cols", bufs=1))
        vp = ctx.enter_context(tc.tile_pool(name="v", bufs=4))
        pp = ctx.enter_context(tc.tile_pool(name="psum", bufs=4, space="PSUM"))
        sp = ctx.enter_context(tc.tile_pool(name="small", bufs=1))

        t01_sb = colp.tile([P, N], F16)
        pn_sb = colp.tile([P, N], F16)
        q_sb = colp.tile([P, N], F16)

        def bcast(dst, ext, lo, hi, piece=4096):
            while lo < hi:
                m = min(lo + piece, hi)
                nc.sync.dma_start(
                    dst[:, lo:m],
                    bass.AP(tensor=ext, offset=lo, ap=[[0, P], [1, m - lo]]),
                )
                lo = m

        # small per-partition tensors first (they gate the first runs and
        # their many tiny DMA packets are cheap to front-run), then column
        # broadcasts in consumption order. Broadcast pieces are kept large
        # (>=2048 cols = 4KB packets) for DMA-queue throughput, with two
        # small lead pieces so the first runs can start early.
        rows_sb = constp.tile([P, NSLOT, 5], F32)
        nc.sync.dma_start(
            rows_sb[:], rows_ext[:, :].rearrange("p (r s) -> p r s", s=5)
        )
        phib = constp.tile([P, NSLOT, K], BF16)
        nc.sync.dma_start(
            phib[:], phib_ext[:, :].rearrange("p (r k) -> p r k", k=K)
        )
        bcast(pn_sb, pn_ext, 4096, 6144, piece=1024)
        bcast(pn_sb, pn_ext, 6144, 8192, piece=2048)
        for lo, hi in plan["t01_ranges"]:
            if hi > 4096:
                bcast(t01_sb, t01_ext, max(lo, 4096), hi, piece=2048)
        for lo, hi in plan["q_ranges"]:
            if hi > 4096:
                bcast(q_sb, q_ext, max(lo, 4096), hi, piece=2048)
        bcast(pn_sb, pn_ext, 0, 4096, piece=2048)
        for lo, hi in plan["t01_ranges"]:
            if lo < 4096:
                bcast(t01_sb, t01_ext, lo, min(hi, 4096), piece=2048)
        for lo, hi in plan["q_ranges"]:
            if lo < 4096:
                bcast(q_sb, q_ext, lo, min(hi, 4096), piece=2048)
        psi = constp.tile([K, N], F32)
        nc.sync.dma_start(psi[:], psi_ext[:, :])
        psiRA = constp.tile([K * 16, 256], F32)
        nc.sync.dma_start(psiRA[:], psiRA_ext[:, :])
        psiRB = constp.tile([K * 16, 128], F32)
        nc.sync.dma_start(psiRB[:], psiRB_ext[:, :])

        acc = sp.tile([K * 16, NACC], F32)
        nc.vector.memset(acc[:], 0.0)
        XsA = sp.tile([K, 4096], F32)
        XsB = sp.tile([K, 2048], F32)
        ttr_scr = sp.tile([K * 16, 256], F32)
        ttr_scr2 = sp.tile([K, CH], F32)

        chunk_tiles = {}

        def get_chunk(c):
            if c not in chunk_tiles:
                chunk_tiles[c] = pp.tile([K, CH], F32, tag="X", name=f"X{c}")
            return chunk_tiles[c]

        # ACT offload set: phase-A far runs of slots 1..7 (slot 0's stay on
        # the faster DVE so the PE pipeline starts quickly)
        act_far = set()
        for k in range(1, NSLOT):
            for kind, a, b in emitA[k]:
                if kind == "far":
                    act_far.add((k, a, b))

        def emit_run(k, kind, a, b, first_t, last_t):
            while b - a > 4096:
                emit_run(k, kind, a, a + 4096, first_t, last_t)
                a += 4096
            w = b - a
            v = vp.tile([P, 4096], BF16, tag="v")
            if kind == "cust":
                nc.vector._custom_dve(
                    ARL_PN,
                    out=v[:, :w],
                    in0=t01_sb[:, a:b],
                    in1=pn_sb[:, a:b],
                    s0=rows_sb[:, k, 0:1],
                    s1=rows_sb[:, k, 1:2],
                    imm2=0.1,
                )
            elif (k, a, b) in act_far:
                nc.scalar.activation(
                    v[:, :w],
                    pn_sb[:, a:b],
                    mybir.ActivationFunctionType.Relu,
                    bias=rows_sb[:, k, 4:5],
                    scale=1.0,
                )
            else:
                src = q_sb if kind == "mid" else pn_sb
                nc.vector.tensor_scalar(
                    v[:, :w],
                    src[:, a:b],
                    rows_sb[:, k, _KIND_SCAL[kind] : _KIND_SCAL[kind] + 1],
                    0.0,
                    AL.add,
                    AL.max,
                )
            c0, c1 = a // CH, (b - 1) // CH
            for c in range(c0, c1 + 1):
                lo, hi = max(a, c * CH), min(b, (c + 1) * CH)
                for m0 in range(lo, hi, 512):
                    m1 = min(m0 + 512, hi)
                    nc.tensor.matmul(
                        get_chunk(c)[:, m0 - c * CH : m1 - c * CH],
                        phib[:, k, :],
                        v[:, m0 - a : m1 - a],
                        start=(k == first_t[c]),
                        stop=(k == last_t[c]),
                    )

        def drain_copy(c, Xs, base):
            nc.scalar.copy(
                Xs[:, c * CH - base : (c + 1) * CH - base], get_chunk(c)[:]
            )

        def drain_direct(c, slot_idx):
            nc.vector._custom_dve(
                dve_ops.TENSOR_TENSOR_REDUCE,
                out=ttr_scr2[:],
                in0=get_chunk(c)[:],
                in1=psi[:, c * CH : (c + 1) * CH],
                s0=0.0,
                s1=1.0,
                accum_out=acc[0:K, slot_idx : slot_idx + 1],
            )

        # ---- phase A: columns [4096, 8192) ----
        drainedA = []
        for k in range(NSLOT):
            for kind, a, b in emitA[k]:
                emit_run(k, kind, a, b, firstA, lastA)
            for c in sorted(lastA):
                if lastA[c] == k:
                    drain_copy(c, XsA, PHASE_SPLIT)
                    drainedA.append(c)
        # reshape [7,4096] -> [112,256] (per-n DMAs); TTR emitted later so
        # the in-order DVE queue does not stall on the drain dependencies.
        xrA = sp.tile([K * 16, 256], F32)
        for n in range(K):
            nc.sync.dma_start(
                xrA[n * 16 : (n + 1) * 16, :],
                XsA[n : n + 1, :].rearrange("o (k f) -> o k f", k=16),
            )

        # ---- phase B: columns [0, 4096) ----
        xrB = sp.tile([K * 16, 128], F32)
        ttr_scrB = sp.tile([K * 16, 128], F32)
        for k in range(NSLOT):
            for kind, a, b in emitB[k]:
                emit_run(k, kind, a, b, firstB, lastB)
            for c in sorted(lastB):
                if lastB[c] == k:
                    if c <= 1:
                        drain_copy(c, XsB, 0)
                    else:
                        drain_direct(c, c)
            if k == 1:
                # xrB reshape can dispatch once chunks 0-1 are copied
                for n in range(K):
                    nc.sync.dma_start(
                        xrB[n * 16 : (n + 1) * 16, :],
                        XsB[n : n + 1, :].rearrange("o (k f) -> o k f", k=16),
                    )
            if k == 2:
                nc.vector._custom_dve(
                    dve_ops.TENSOR_TENSOR_REDUCE,
                    out=ttr_scr[:],
                    in0=xrA[:],
                    in1=psiRA[:],
                    s0=0.0,
                    s1=1.0,
                    accum_out=acc[:, 0:1],
                )
                nc.vector._custom_dve(
                    dve_ops.TENSOR_TENSOR_REDUCE,
                    out=ttr_scrB[:],
                    in0=xrB[:],
                    in1=psiRB[:],
                    s0=0.0,
                    s1=1.0,
                    accum_out=acc[:, 1:2],
                )

        nc.sync.dma_start(
            out_ext[:].rearrange("(p c) -> p c", c=NACC), acc[:]
        )

    nc.compile()
    return nc


_NC_CACHE = {}


def _get_nc(plan, key):
    if key not in _NC_CACHE:
        _NC_CACHE[key] = _build_nc(plan)
    return _NC_CACHE[key]


def _exact_count(t: np.ndarray) -> int:
    n = t.shape[0]
    _, cnts = np.unique(t, return_counts=True)
    dup = int(sum(int(c) * (int(c) - 1) // 2 for c in cnts[cnts > 1]))
    return n * (n - 1) // 2 - dup


def _make_in_maps(predictions, targets, uncertainties):
    import ml_dtypes

    t = np.ascontiguousarray(np.asarray(targets, np.float32))
    p = np.ascontiguousarray(np.asarray(predictions, np.float32))
    u = np.ascontiguousarray(np.asarray(uncertainties, np.float32))
    order = np.argsort(t, kind="stable")
    ts, ps, us = t[order], p[order], u[order]
    ts64, ps64 = ts.astype(np.float64), ps.astype(np.float64)

    t01_h = (0.1 * ts64).astype(np.float16)
    pn_h = (-ps64).astype(np.float16)
    q_h = (0.1 * ts64 - ps64).astype(np.float16)

    # Psi[n, j] = x_j^n, x = u - 0.5
    x = us.astype(np.float64) - 0.5
    psi = np.stack([x**n for n in range(K)]).astype(np.float32)  # [K, N]
    psiRA = np.ascontiguousarray(
        psi[:, PHASE_SPLIT:].reshape(K, 16, 256).reshape(K * 16, 256)
    )
    psiRB = np.ascontiguousarray(
        psi[:, 0:2048].reshape(K, 16, 128).reshape(K * 16, 128)
    )

    # Phi[i, n] = sum_m A[m, n] x_i^m  (float64 -> bf16)
    xp = np.stack([x**m for m in range(K)])  # [K, N]
    phi = np.einsum("mn,mj->jn", _ACOEF, xp)  # [N, K]

    in_maps = []
    for c in range(NCORES):
        pos = (np.arange(NSLOT)[:, None] * 1024 + 8 * np.arange(P)[None, :] + c)
        # rows5[p, slot, s]
        rows5 = np.zeros((P, NSLOT, 5), np.float64)
        tp = ts64[pos]  # [slot, p]
        pp_ = ps64[pos]
        rows5[:, :, 0] = (0.1 * tp).T
        rows5[:, :, 1] = pp_.T
        rows5[:, :, 2] = (-(0.1 * tp - pp_)).T
        rows5[:, :, 3] = (pp_ + 0.01).T
        rows5[:, :, 4] = (pp_ + 0.1).T
        phib = phi[pos, :]  # [slot, p, K]
        phib = np.ascontiguousarray(np.transpose(phib, (1, 0, 2)))  # [p, slot, K]
        in_maps.append(
            {
                "t01col": t01_h,
                "pncol": pn_h,
                "qcol": q_h,
                "rows5": np.ascontiguousarray(
                    rows5.reshape(P, 5 * NSLOT)
                ).astype(np.float32),
                "phib": phib.reshape(P, NSLOT * K).astype(ml_dtypes.bfloat16),
                "psi": psi,
                "psiRA": psiRA,
                "psiRB": psiRB,
            }
        )
    plan = _make_plan(ts)
    return in_maps, t, plan


def _run_device(in_maps, plan, plan_key, trace=False, **kw):
    nc = _get_nc(plan, plan_key)
    return run_bass_kernel_spmd(
        nc, in_maps, core_ids=list(range(NCORES)), trace=trace, **kw
    )


def _plan_key(plan):
    return (
        tuple(tuple(r) for k in range(NSLOT) for r in plan["emitA"][k]),
        tuple(tuple(r) for k in range(NSLOT) for r in plan["emitB"][k]),
    )


def kernel(predictions, targets, uncertainties):
    in_maps, t, plan = _make_in_maps(predictions, targets, uncertainties)
    res = _run_device(in_maps, plan, _plan_key(plan))
    total = np.float64(0.0)
    for r in res.results:
        total += np.asarray(r["out"], np.float64).sum()
    count = _exact_count(t)
    return np.asarray(total / max(count, 1), dtype=np.float32)
